# revision 3
# baseline (speedup 1.0000x reference)
"""Causal single-head attention (B=4, T=2048, E=1024, D=128) on 8 TRN2 cores.

Sharding: core c = (b, h) with b = c // 2, h = c % 2. Each core handles batch b
and 4 query "slots" i=0..3: queries [512*i + 256*h, +256), keys [0, 512*(i+1))
(rectangularized causal; exact causality restored via multiplicative masks).
All cores run ONE identical bass program; per-core differences (h-dependent
query columns, RoPE tables, masks) are expressed purely via host-prepared
DRAM input data.

Pipeline per core (all matmuls float32r):
  1. K/V projections over all 2048 tokens from host-pre-transposed xT
     (contraction dim e on partitions), plus a second "swapped" K projection
     with host-row-swapped weights for RoPE pair rotation.
  2. RoPE on DVE: k' = k*cosT + kswap*sinT (tables in [d,t] layout).
  3. V^T -> V natural via PE transposes.
  4. Per slot: S^T chunks = k'^T_chunk.T @ q'^T  -> exp on ACT -> mask ->
     ones-matmul denominator + AV matmul accumulate in one PSUM tile ->
     reciprocal+normalize on DVE -> PE transpose -> out.
"""

import sys

for _p in ("/opt/trn_rl_repo",):
    if _p not in sys.path:
        sys.path.insert(0, _p)

import numpy as np

import concourse.bacc as bacc
import concourse.mybir as mybir
import concourse.tile as tile
from concourse.bass_utils import run_bass_kernel_spmd
from concourse.masks import make_identity

F32 = mybir.dt.float32
F32R = mybir.dt.float32r

B, T, E, D = 4, 2048, 1024, 128
THETA = 10000.0
SCALE = 1.0 / np.sqrt(np.float32(D))
N_CORES = 8
N_SLOTS = 4         # query slots per core, 256 queries each
SLOT_Q = 256
KV_CH = T // 128    # 16 key chunks of 128
N_TC = T // 512     # 512-token chunks for kv projection
N_EC = E // 128     # contraction chunks


def _build_nc():
    nc = bacc.Bacc(None, target_bir_lowering=False, debug=False)

    xT = nc.dram_tensor("xT", [E, T], F32R, kind="ExternalInput")
    xq = nc.dram_tensor("xq", [E, N_SLOTS * SLOT_Q], F32R, kind="ExternalInput")
    wq = nc.dram_tensor("wqT", [E, D], F32R, kind="ExternalInput")
    wqs = nc.dram_tensor("wqswT", [E, D], F32R, kind="ExternalInput")
    wk = nc.dram_tensor("wkT", [E, D], F32R, kind="ExternalInput")
    wks = nc.dram_tensor("wkswT", [E, D], F32R, kind="ExternalInput")
    wv = nc.dram_tensor("wvT", [E, D], F32R, kind="ExternalInput")
    ctabK = nc.dram_tensor("ctabK", [D, T], F32, kind="ExternalInput")
    stabK = nc.dram_tensor("stabK", [D, T], F32, kind="ExternalInput")
    ctabQ = nc.dram_tensor("ctabQ", [D, N_SLOTS * SLOT_Q], F32, kind="ExternalInput")
    stabQ = nc.dram_tensor("stabQ", [D, N_SLOTS * SLOT_Q], F32, kind="ExternalInput")
    masks = nc.dram_tensor("masks", [4, 128, SLOT_Q], F32, kind="ExternalInput")
    ones_d = nc.dram_tensor("ones", [128, 128], F32R, kind="ExternalInput")
    out_d = nc.dram_tensor("out", [N_SLOTS * SLOT_Q, D], F32, kind="ExternalOutput")

    with tile.TileContext(nc) as tc:
        with (
            tc.tile_pool(name="const", bufs=1) as const,
            tc.tile_pool(name="persist", bufs=1) as persist,
            tc.tile_pool(name="xstream", bufs=2) as xstream,
            tc.tile_pool(name="work", bufs=3) as work,
            tc.tile_pool(name="pp", bufs=1, space="PSUM") as pp,
            tc.tile_pool(name="ps", bufs=3, space="PSUM") as ps,
            tc.tile_pool(name="pa", bufs=1, space="PSUM") as pa,
        ):
            ident = const.tile([128, 128], F32)
            make_identity(nc, ident)
            ones = const.tile([128, 128], F32R)
            nc.sync.dma_start(out=ones, in_=ones_d[:])

            w_sb = {}
            for name, dram in (("q", wq), ("qs", wqs), ("k", wk), ("ks", wks), ("v", wv)):
                t = const.tile([128, N_EC, D], F32R, tag=f"w_{name}")
                nc.sync.dma_start(out=t, in_=dram[:].rearrange("(ec p) d -> p ec d", p=128))
                w_sb[name] = t

            ctabK_sb = const.tile([D, T], F32)
            nc.sync.dma_start(out=ctabK_sb, in_=ctabK[:])
            stabK_sb = const.tile([D, T], F32)
            nc.sync.dma_start(out=stabK_sb, in_=stabK[:])
            ctabQ_sb = const.tile([D, N_SLOTS * SLOT_Q], F32)
            nc.sync.dma_start(out=ctabQ_sb, in_=ctabQ[:])
            stabQ_sb = const.tile([D, N_SLOTS * SLOT_Q], F32)
            nc.sync.dma_start(out=stabQ_sb, in_=stabQ[:])
            mask_sb = const.tile([128, 4, SLOT_Q], F32)
            nc.sync.dma_start(out=mask_sb, in_=masks[:].rearrange("j p y -> p j y"))

            kT_sb = persist.tile([D, T], F32R)             # rotated K^T
            qT_sb = persist.tile([D, N_SLOTS * SLOT_Q], F32R)  # rotated Q^T
            v_nat = persist.tile([128, KV_CH, D], F32R)    # V natural, per chunk

            # ---- K / V projections over all tokens (512-token chunks) ----
            for tci in range(N_TC):
                cs = slice(tci * 512, (tci + 1) * 512)
                xt = xstream.tile([128, N_EC, 512], F32R, tag="xt")
                nc.sync.dma_start(out=xt, in_=xT[:, cs].rearrange("(ec p) t -> p ec t", p=128))
                psk = pp.tile([128, 512], F32, tag="psk")
                psks = pp.tile([128, 512], F32, tag="psks")
                psv = pp.tile([128, 512], F32, tag="psv")
                for ec in range(N_EC):
                    st, sp = ec == 0, ec == N_EC - 1
                    nc.tensor.matmul(psk, w_sb["k"][:, ec, :], xt[:, ec, :], start=st, stop=sp)
                    nc.tensor.matmul(psks, w_sb["ks"][:, ec, :], xt[:, ec, :], start=st, stop=sp)
                    nc.tensor.matmul(psv, w_sb["v"][:, ec, :], xt[:, ec, :], start=st, stop=sp)
                t1 = work.tile([128, 512], F32, tag="ropeA")
                nc.vector.tensor_mul(t1, psk, ctabK_sb[:, cs])
                t2 = work.tile([128, 512], F32, tag="ropeB")
                nc.vector.tensor_mul(t2, psks, stabK_sb[:, cs])
                nc.vector.tensor_add(kT_sb[:, cs], t1, t2)
                vt = work.tile([128, 512], F32, tag="vt")
                nc.vector.tensor_copy(vt, psv)
                for j in range(4):
                    pt = ps.tile([128, 128], F32, tag="s")
                    nc.tensor.transpose(pt, vt[:, j * 128:(j + 1) * 128], ident)
                    nc.scalar.copy(v_nat[:, tci * 4 + j, :], pt)

            # ---- Q projection (4 slots of 256 queries) ----
            for si in range(N_SLOTS):
                qs = slice(si * SLOT_Q, (si + 1) * SLOT_Q)
                xtq = xstream.tile([128, N_EC, SLOT_Q], F32R, tag="xtq")
                nc.sync.dma_start(out=xtq, in_=xq[:, qs].rearrange("(ec p) t -> p ec t", p=128))
                psq = pp.tile([128, SLOT_Q], F32, tag="psk")
                psqs = pp.tile([128, SLOT_Q], F32, tag="psks")
                for ec in range(N_EC):
                    st, sp = ec == 0, ec == N_EC - 1
                    nc.tensor.matmul(psq, w_sb["q"][:, ec, :], xtq[:, ec, :], start=st, stop=sp)
                    nc.tensor.matmul(psqs, w_sb["qs"][:, ec, :], xtq[:, ec, :], start=st, stop=sp)
                t1 = work.tile([128, SLOT_Q], F32, tag="qropeA")
                nc.vector.tensor_mul(t1, psq, ctabQ_sb[:, qs])
                t2 = work.tile([128, SLOT_Q], F32, tag="qropeB")
                nc.vector.tensor_mul(t2, psqs, stabQ_sb[:, qs])
                nc.vector.tensor_add(qT_sb[:, qs], t1, t2)

            # ---- Attention ----
            for si in range(N_SLOTS):
                qs = slice(si * SLOT_Q, (si + 1) * SLOT_Q)
                n_ch = 4 * (si + 1)
                pacc_av = pa.tile([128, SLOT_Q], F32, tag="pacc_av")
                pacc_d = pa.tile([128, SLOT_Q], F32, tag="pacc_d")
                for c in range(n_ch):
                    pss = ps.tile([128, SLOT_Q], F32, tag="s")
                    nc.tensor.matmul(pss, kT_sb[:, c * 128:(c + 1) * 128], qT_sb[:, qs],
                                     start=True, stop=True)
                    pT = work.tile([128, SLOT_Q], F32R, tag="pT")
                    nc.scalar.activation(out=pT, in_=pss,
                                         func=mybir.ActivationFunctionType.Exp, scale=float(SCALE))
                    j = c - (n_ch - 4)
                    if j >= 0:
                        nc.vector.tensor_mul(pT, pT, mask_sb[:, j, :])
                    st, sp = c == 0, c == n_ch - 1
                    nc.tensor.matmul(pacc_d, ones, pT, start=st, stop=sp)
                    nc.tensor.matmul(pacc_av, v_nat[:, c, :], pT, start=st, stop=sp)
                recip = work.tile([128, SLOT_Q], F32, tag="recip")
                nc.vector.reciprocal(recip, pacc_d)
                oT = work.tile([128, SLOT_Q], F32, tag="oT")
                nc.vector.tensor_mul(oT, pacc_av, recip)
                for j in range(2):
                    pt = ps.tile([128, 128], F32, tag="s")
                    nc.tensor.transpose(pt, oT[:, j * 128:(j + 1) * 128], ident)
                    on = work.tile([128, 128], F32, tag="onat")
                    nc.scalar.copy(on, pt)
                    nc.sync.dma_start(out=out_d[si * SLOT_Q + j * 128: si * SLOT_Q + (j + 1) * 128, :],
                                      in_=on)
    nc.compile()
    return nc


_NC = None


def _get_nc():
    global _NC
    if _NC is None:
        _NC = _build_nc()
    return _NC


def _host_prep(embedding_word, w_Q, w_K, w_V):
    x = np.asarray(embedding_word, dtype=np.float32)
    w_Q = np.asarray(w_Q, dtype=np.float32)
    w_K = np.asarray(w_K, dtype=np.float32)
    w_V = np.asarray(w_V, dtype=np.float32)

    swap = np.arange(D).reshape(-1, 2)[:, ::-1].reshape(-1)  # [1,0,3,2,...]

    wqT = np.ascontiguousarray(w_Q.T)
    wkT = np.ascontiguousarray(w_K.T)
    wvT = np.ascontiguousarray(w_V.T)
    wqsT = np.ascontiguousarray(w_Q[swap].T)
    wksT = np.ascontiguousarray(w_K[swap].T)

    # RoPE tables in [d, t] layout
    j = np.arange(D // 2, dtype=np.float64)
    freqs = 1.0 / THETA ** (2.0 * j / D)          # theta_j
    t = np.arange(T, dtype=np.float64)
    ang = np.outer(freqs, t)                      # [64, T]
    cos = np.cos(ang)
    sin = np.sin(ang)
    ctab = np.repeat(cos, 2, axis=0).astype(np.float32)           # [128, T]
    stab = np.empty((D, T), dtype=np.float32)
    stab[0::2] = -sin
    stab[1::2] = sin

    xT_all = [np.ascontiguousarray(x[b].T) for b in range(B)]

    qcols = {}
    for h in (0, 1):
        cols = np.concatenate([np.arange(512 * i + 256 * h, 512 * i + 256 * h + SLOT_Q)
                               for i in range(N_SLOTS)])
        qcols[h] = cols

    yy, xx = np.meshgrid(np.arange(SLOT_Q), np.arange(128), indexing="xy")
    # mask_j[x, y] = 1 if (y - x) >= 128*j - 256*h
    masks_h = {}
    for h in (0, 1):
        m = np.empty((4, 128, SLOT_Q), dtype=np.float32)
        for jj in range(4):
            xg, yg = np.meshgrid(np.arange(128), np.arange(SLOT_Q), indexing="ij")
            m[jj] = ((yg - xg) >= (128 * jj - 256 * h)).astype(np.float32)
        masks_h[h] = m

    ones = np.ones((128, 128), dtype=np.float32)

    in_maps = []
    for c in range(N_CORES):
        b, h = c // 2, c % 2
        cols = qcols[h]
        in_maps.append({
            "xT": xT_all[b],
            "xq": np.ascontiguousarray(xT_all[b][:, cols]),
            "wqT": wqT, "wqswT": wqsT, "wkT": wkT, "wkswT": wksT, "wvT": wvT,
            "ctabK": ctab, "stabK": stab,
            "ctabQ": np.ascontiguousarray(ctab[:, cols]),
            "stabQ": np.ascontiguousarray(stab[:, cols]),
            "masks": masks_h[h],
            "ones": ones,
        })
    return in_maps


def _assemble(results):
    out = np.empty((B, T, D), dtype=np.float32)
    for c in range(N_CORES):
        b, h = c // 2, c % 2
        o = results[c]["out"]  # [1024, 128]
        for i in range(N_SLOTS):
            out[b, 512 * i + 256 * h: 512 * i + 256 * h + SLOT_Q, :] = \
                o[i * SLOT_Q:(i + 1) * SLOT_Q, :]
    return out


def run(inputs, trace=False, tmpdir=None):
    nc = _get_nc()
    in_maps = _host_prep(**inputs)
    res = run_bass_kernel_spmd(nc, in_maps, list(range(N_CORES)), trace=trace, tmpdir=tmpdir)
    return _assemble(res.results), res


def kernel(embedding_word, w_Q, w_K, w_V):
    out, _ = run(dict(embedding_word=embedding_word, w_Q=w_Q, w_K=w_K, w_V=w_V))
    return out


# revision 4
# speedup vs baseline: 1.0232x; 1.0232x over previous
"""Causal single-head attention (B=4, T=2048, E=1024, D=128) on 8 TRN2 cores.

Sharding: core c = (b, h) with b = c // 2, h = c % 2. Each core handles batch b
and 4 query "slots" i=0..3: queries [512*i + 256*h, +256), keys [0, 512*(i+1))
(rectangularized causal; exact causality restored via multiplicative masks).
All cores run ONE identical bass program; per-core differences (h-dependent
query columns, RoPE tables, masks) are expressed purely via host-prepared
DRAM input data.

Pipeline per core (all matmuls float32r):
  1. K/V projections over all 2048 tokens from host-pre-transposed xT
     (contraction dim e on partitions), plus a second "swapped" K projection
     with host-row-swapped weights for RoPE pair rotation.
  2. RoPE on DVE: k' = k*cosT + kswap*sinT (tables in [d,t] layout).
  3. V^T -> V natural via PE transposes.
  4. Per slot: S^T chunks = k'^T_chunk.T @ q'^T  -> exp on ACT -> mask ->
     ones-matmul denominator + AV matmul accumulate in one PSUM tile ->
     reciprocal+normalize on DVE -> PE transpose -> out.
"""

import sys

for _p in ("/opt/trn_rl_repo",):
    if _p not in sys.path:
        sys.path.insert(0, _p)

import numpy as np

import concourse.bacc as bacc
import concourse.mybir as mybir
import concourse.tile as tile
from concourse.bass_utils import run_bass_kernel_spmd
from concourse.masks import make_identity

F32 = mybir.dt.float32
F32R = mybir.dt.float32r

B, T, E, D = 4, 2048, 1024, 128
THETA = 10000.0
SCALE = 1.0 / np.sqrt(np.float32(D))
N_CORES = 8
N_SLOTS = 4         # query slots per core, 256 queries each
SLOT_Q = 256
KV_CH = T // 128    # 16 key chunks of 128
N_TC = T // 512     # 512-token chunks for kv projection
N_EC = E // 128     # contraction chunks


def _build_nc():
    nc = bacc.Bacc(None, target_bir_lowering=False, debug=False)

    xT = nc.dram_tensor("xT", [E, T], F32R, kind="ExternalInput")
    xq = nc.dram_tensor("xq", [E, N_SLOTS * SLOT_Q], F32R, kind="ExternalInput")
    wq = nc.dram_tensor("wqT", [E, D], F32R, kind="ExternalInput")
    wqs = nc.dram_tensor("wqswT", [E, D], F32R, kind="ExternalInput")
    wk = nc.dram_tensor("wkT", [E, D], F32R, kind="ExternalInput")
    wks = nc.dram_tensor("wkswT", [E, D], F32R, kind="ExternalInput")
    wv = nc.dram_tensor("wvT", [E, D], F32R, kind="ExternalInput")
    ctabK = nc.dram_tensor("ctabK", [D, T], F32, kind="ExternalInput")
    stabK = nc.dram_tensor("stabK", [D, T], F32, kind="ExternalInput")
    ctabQ = nc.dram_tensor("ctabQ", [D, N_SLOTS * SLOT_Q], F32, kind="ExternalInput")
    stabQ = nc.dram_tensor("stabQ", [D, N_SLOTS * SLOT_Q], F32, kind="ExternalInput")
    masks = nc.dram_tensor("masks", [4, 128, SLOT_Q], F32, kind="ExternalInput")
    ones_d = nc.dram_tensor("ones", [128, 128], F32R, kind="ExternalInput")
    out_d = nc.dram_tensor("out", [N_SLOTS * SLOT_Q, D], F32, kind="ExternalOutput")

    with tile.TileContext(nc) as tc:
        with (
            tc.tile_pool(name="const", bufs=1) as const,
            tc.tile_pool(name="persist", bufs=1) as persist,
            tc.tile_pool(name="xstream", bufs=2) as xstream,
            tc.tile_pool(name="work", bufs=4) as work,
            tc.tile_pool(name="pp", bufs=1, space="PSUM") as pp,
            tc.tile_pool(name="ps", bufs=3, space="PSUM") as ps,
            tc.tile_pool(name="pa", bufs=1, space="PSUM") as pa,
        ):
            ident = const.tile([128, 128], F32)
            make_identity(nc, ident)
            ones = const.tile([128, 128], F32R)
            nc.scalar.dma_start(out=ones, in_=ones_d[:])

            w_sb = {}
            for name, dram in (("q", wq), ("qs", wqs), ("k", wk), ("ks", wks), ("v", wv)):
                t = const.tile([128, N_EC, D], F32R, tag=f"w_{name}")
                nc.scalar.dma_start(out=t, in_=dram[:].rearrange("(ec p) d -> p ec d", p=128))
                w_sb[name] = t

            ctabK_sb = const.tile([D, T], F32)
            nc.gpsimd.dma_start(out=ctabK_sb, in_=ctabK[:])
            stabK_sb = const.tile([D, T], F32)
            nc.gpsimd.dma_start(out=stabK_sb, in_=stabK[:])
            ctabQ_sb = const.tile([D, N_SLOTS * SLOT_Q], F32)
            nc.gpsimd.dma_start(out=ctabQ_sb, in_=ctabQ[:])
            stabQ_sb = const.tile([D, N_SLOTS * SLOT_Q], F32)
            nc.gpsimd.dma_start(out=stabQ_sb, in_=stabQ[:])
            mask_sb = const.tile([128, 4, SLOT_Q], F32)
            nc.gpsimd.dma_start(out=mask_sb, in_=masks[:].rearrange("j p y -> p j y"))

            kT_sb = persist.tile([D, T], F32R)             # rotated K^T
            qT_sb = persist.tile([D, N_SLOTS * SLOT_Q], F32R)  # rotated Q^T
            v_nat = persist.tile([128, KV_CH, D], F32R)    # V natural, per chunk

            # ---- K / V projections over all tokens (512-token chunks) ----
            for tci in range(N_TC):
                cs = slice(tci * 512, (tci + 1) * 512)
                xt = xstream.tile([128, N_EC, 512], F32R, tag="xt")
                xTr = xT[:, cs].rearrange("(ec p) t -> p ec t", p=128)
                for ec in range(N_EC):
                    nc.sync.dma_start(out=xt[:, ec, :], in_=xTr[:, ec, :])
                psk = pp.tile([128, 512], F32, tag="psk")
                psks = pp.tile([128, 512], F32, tag="psks")
                psv = pp.tile([128, 512], F32, tag="psv")
                for ec in range(N_EC):
                    st, sp = ec == 0, ec == N_EC - 1
                    nc.tensor.matmul(psk, w_sb["k"][:, ec, :], xt[:, ec, :], start=st, stop=sp)
                    nc.tensor.matmul(psks, w_sb["ks"][:, ec, :], xt[:, ec, :], start=st, stop=sp)
                    nc.tensor.matmul(psv, w_sb["v"][:, ec, :], xt[:, ec, :], start=st, stop=sp)
                t1 = work.tile([128, 512], F32, tag="ropeA")
                nc.vector.tensor_mul(t1, psk, ctabK_sb[:, cs])
                t2 = work.tile([128, 512], F32, tag="ropeB")
                nc.vector.tensor_mul(t2, psks, stabK_sb[:, cs])
                nc.vector.tensor_add(kT_sb[:, cs], t1, t2)
                vt = work.tile([128, 512], F32, tag="vt")
                nc.vector.tensor_copy(vt, psv)
                for j in range(4):
                    pt = ps.tile([128, 128], F32, tag="s")
                    nc.tensor.transpose(pt, vt[:, j * 128:(j + 1) * 128], ident)
                    nc.scalar.copy(v_nat[:, tci * 4 + j, :], pt)

            # ---- Q projection (4 slots of 256 queries) ----
            for si in range(N_SLOTS):
                qs = slice(si * SLOT_Q, (si + 1) * SLOT_Q)
                xtq = xstream.tile([128, N_EC, SLOT_Q], F32R, tag="xtq")
                nc.gpsimd.dma_start(out=xtq, in_=xq[:, qs].rearrange("(ec p) t -> p ec t", p=128))
                psq = pp.tile([128, SLOT_Q], F32, tag="psk")
                psqs = pp.tile([128, SLOT_Q], F32, tag="psks")
                for ec in range(N_EC):
                    st, sp = ec == 0, ec == N_EC - 1
                    nc.tensor.matmul(psq, w_sb["q"][:, ec, :], xtq[:, ec, :], start=st, stop=sp)
                    nc.tensor.matmul(psqs, w_sb["qs"][:, ec, :], xtq[:, ec, :], start=st, stop=sp)
                t1 = work.tile([128, SLOT_Q], F32, tag="qropeA")
                nc.vector.tensor_mul(t1, psq, ctabQ_sb[:, qs])
                t2 = work.tile([128, SLOT_Q], F32, tag="qropeB")
                nc.vector.tensor_mul(t2, psqs, stabQ_sb[:, qs])
                nc.vector.tensor_add(qT_sb[:, qs], t1, t2)

            # ---- Attention ----
            for si in range(N_SLOTS):
                qs = slice(si * SLOT_Q, (si + 1) * SLOT_Q)
                n_ch = 4 * (si + 1)
                pacc_av = pa.tile([128, SLOT_Q], F32, tag="pacc_av")
                pacc_d = pa.tile([128, SLOT_Q], F32, tag="pacc_d")
                for c in range(n_ch):
                    pss = ps.tile([128, SLOT_Q], F32, tag="s")
                    nc.tensor.matmul(pss, kT_sb[:, c * 128:(c + 1) * 128], qT_sb[:, qs],
                                     start=True, stop=True)
                    pT = work.tile([128, SLOT_Q], F32R, tag="pT")
                    nc.scalar.activation(out=pT, in_=pss,
                                         func=mybir.ActivationFunctionType.Exp, scale=float(SCALE))
                    j = c - (n_ch - 4)
                    if j >= 0:
                        nc.vector.tensor_mul(pT, pT, mask_sb[:, j, :])
                    st, sp = c == 0, c == n_ch - 1
                    nc.tensor.matmul(pacc_d, ones, pT, start=st, stop=sp)
                    nc.tensor.matmul(pacc_av, v_nat[:, c, :], pT, start=st, stop=sp)
                recip = work.tile([128, SLOT_Q], F32, tag="recip")
                nc.vector.reciprocal(recip, pacc_d)
                oT = work.tile([128, SLOT_Q], F32, tag="oT")
                nc.vector.tensor_mul(oT, pacc_av, recip)
                for j in range(2):
                    pt = pp.tile([128, 128], F32, tag="psv")
                    nc.tensor.transpose(pt, oT[:, j * 128:(j + 1) * 128], ident)
                    on = work.tile([128, 128], F32, tag="onat")
                    nc.scalar.copy(on, pt)
                    nc.sync.dma_start(out=out_d[si * SLOT_Q + j * 128: si * SLOT_Q + (j + 1) * 128, :],
                                      in_=on)
    nc.compile()
    return nc


_NC = None


def _get_nc():
    global _NC
    if _NC is None:
        _NC = _build_nc()
    return _NC


def _host_prep(embedding_word, w_Q, w_K, w_V):
    x = np.asarray(embedding_word, dtype=np.float32)
    w_Q = np.asarray(w_Q, dtype=np.float32)
    w_K = np.asarray(w_K, dtype=np.float32)
    w_V = np.asarray(w_V, dtype=np.float32)

    swap = np.arange(D).reshape(-1, 2)[:, ::-1].reshape(-1)  # [1,0,3,2,...]

    wqT = np.ascontiguousarray(w_Q.T)
    wkT = np.ascontiguousarray(w_K.T)
    wvT = np.ascontiguousarray(w_V.T)
    wqsT = np.ascontiguousarray(w_Q[swap].T)
    wksT = np.ascontiguousarray(w_K[swap].T)

    # RoPE tables in [d, t] layout
    j = np.arange(D // 2, dtype=np.float64)
    freqs = 1.0 / THETA ** (2.0 * j / D)          # theta_j
    t = np.arange(T, dtype=np.float64)
    ang = np.outer(freqs, t)                      # [64, T]
    cos = np.cos(ang)
    sin = np.sin(ang)
    ctab = np.repeat(cos, 2, axis=0).astype(np.float32)           # [128, T]
    stab = np.empty((D, T), dtype=np.float32)
    stab[0::2] = -sin
    stab[1::2] = sin

    xT_all = [np.ascontiguousarray(x[b].T) for b in range(B)]

    qcols = {}
    for h in (0, 1):
        cols = np.concatenate([np.arange(512 * i + 256 * h, 512 * i + 256 * h + SLOT_Q)
                               for i in range(N_SLOTS)])
        qcols[h] = cols

    yy, xx = np.meshgrid(np.arange(SLOT_Q), np.arange(128), indexing="xy")
    # mask_j[x, y] = 1 if (y - x) >= 128*j - 256*h
    masks_h = {}
    for h in (0, 1):
        m = np.empty((4, 128, SLOT_Q), dtype=np.float32)
        for jj in range(4):
            xg, yg = np.meshgrid(np.arange(128), np.arange(SLOT_Q), indexing="ij")
            m[jj] = ((yg - xg) >= (128 * jj - 256 * h)).astype(np.float32)
        masks_h[h] = m

    ones = np.ones((128, 128), dtype=np.float32)

    in_maps = []
    for c in range(N_CORES):
        b, h = c // 2, c % 2
        cols = qcols[h]
        in_maps.append({
            "xT": xT_all[b],
            "xq": np.ascontiguousarray(xT_all[b][:, cols]),
            "wqT": wqT, "wqswT": wqsT, "wkT": wkT, "wkswT": wksT, "wvT": wvT,
            "ctabK": ctab, "stabK": stab,
            "ctabQ": np.ascontiguousarray(ctab[:, cols]),
            "stabQ": np.ascontiguousarray(stab[:, cols]),
            "masks": masks_h[h],
            "ones": ones,
        })
    return in_maps


def _assemble(results):
    out = np.empty((B, T, D), dtype=np.float32)
    for c in range(N_CORES):
        b, h = c // 2, c % 2
        o = results[c]["out"]  # [1024, 128]
        for i in range(N_SLOTS):
            out[b, 512 * i + 256 * h: 512 * i + 256 * h + SLOT_Q, :] = \
                o[i * SLOT_Q:(i + 1) * SLOT_Q, :]
    return out


def run(inputs, trace=False, tmpdir=None):
    nc = _get_nc()
    in_maps = _host_prep(**inputs)
    res = run_bass_kernel_spmd(nc, in_maps, list(range(N_CORES)), trace=trace, tmpdir=tmpdir)
    return _assemble(res.results), res


def kernel(embedding_word, w_Q, w_K, w_V):
    out, _ = run(dict(embedding_word=embedding_word, w_Q=w_Q, w_K=w_K, w_V=w_V))
    return out


# revision 6
# speedup vs baseline: 1.0755x; 1.0511x over previous
"""Causal single-head attention (B=4, T=2048, E=1024, D=128) on 8 TRN2 cores.

Sharding: core c = (b, h) with b = c // 2, h = c % 2. Each core handles batch b
and 4 query "slots" i=0..3: queries [512*i + 256*h, +256), keys [0, 512*(i+1))
(rectangularized causal; exact causality via data-driven multiplicative masks).
All cores run ONE identical bass program; per-core differences are expressed
purely via host-prepared DRAM input data.

Per core (all matmuls float32r):
  1. K/V projections over all 2048 tokens from host-pre-transposed, pre-packed
     xT tiles (contraction dim e on partitions, fully contiguous DMA).
  2. RoPE: raw k evicted to SBUF, partition-pair-swapped via 2 stride-2
     SBUF->SBUF DMAs, combined on DVE: k' = k*cosT + kswap*sinT.
  3. V^T -> V natural via PE transposes.
  4. Per slot: S^T chunk = k'^T_chunk.T @ q'^T -> exp on ACT -> mask mul ->
     ones-matmul denominator + AV matmul (separate PSUM banks) ->
     reciprocal+normalize on DVE -> PE transpose -> out.
"""

import sys

for _p in ("/opt/trn_rl_repo",):
    if _p not in sys.path:
        sys.path.insert(0, _p)

import numpy as np

import concourse.bacc as bacc
import concourse.mybir as mybir
import concourse.tile as tile
from concourse.bass_utils import run_bass_kernel_spmd
from concourse.masks import make_identity

F32 = mybir.dt.float32
F32R = mybir.dt.float32r

B, T, E, D = 4, 2048, 1024, 128
THETA = 10000.0
SCALE = 1.0 / np.sqrt(np.float32(D))
N_CORES = 8
N_SLOTS = 4
SLOT_Q = 256
KV_CH = T // 128
N_TC = T // 512
N_EC = E // 128


def _build_nc():
    nc = bacc.Bacc(None, target_bir_lowering=False, debug=False)

    # pre-packed inputs: [partition, ...] layouts, fully contiguous per row
    wk = nc.dram_tensor("wk", [128, N_EC, D], F32R, kind="ExternalInput")
    wv = nc.dram_tensor("wv", [128, N_EC, D], F32R, kind="ExternalInput")
    wq = nc.dram_tensor("wq", [128, N_EC, D], F32R, kind="ExternalInput")
    xt_d = nc.dram_tensor("xt", [128, N_TC, N_EC, 512], F32R, kind="ExternalInput")
    xq_d = nc.dram_tensor("xq", [128, N_SLOTS, N_EC, SLOT_Q], F32R, kind="ExternalInput")
    ctabK = nc.dram_tensor("ctabK", [D, T], F32, kind="ExternalInput")
    stabK = nc.dram_tensor("stabK", [D, T], F32, kind="ExternalInput")
    ctabQ = nc.dram_tensor("ctabQ", [D, N_SLOTS * SLOT_Q], F32, kind="ExternalInput")
    stabQ = nc.dram_tensor("stabQ", [D, N_SLOTS * SLOT_Q], F32, kind="ExternalInput")
    masks = nc.dram_tensor("masks", [128, 4, SLOT_Q], F32, kind="ExternalInput")
    ones_d = nc.dram_tensor("ones", [128, 128], F32R, kind="ExternalInput")
    out_d = nc.dram_tensor("out", [N_SLOTS * SLOT_Q, D], F32, kind="ExternalOutput")

    with tile.TileContext(nc) as tc:
        with (
            tc.tile_pool(name="const", bufs=1) as const,
            tc.tile_pool(name="persist", bufs=1) as persist,
            tc.tile_pool(name="work", bufs=2) as work,
            tc.tile_pool(name="pp", bufs=1, space="PSUM") as pp,
            tc.tile_pool(name="ps", bufs=3, space="PSUM") as ps,
            tc.tile_pool(name="pa", bufs=1, space="PSUM") as pa,
        ):
            # --- weights first (scalar queue), xt (sync), rest on gpsimd ---
            w_sb = {}
            for name, dram in (("k", wk), ("v", wv), ("q", wq)):
                t = const.tile([128, N_EC, D], F32R, tag=f"w_{name}")
                nc.scalar.dma_start(out=t, in_=dram[:])
                w_sb[name] = t

            xt = persist.tile([128, N_TC, N_EC, 512], F32R)
            for tci in range(N_TC):
                nc.sync.dma_start(out=xt[:, tci], in_=xt_d[:, tci])

            ctabK_sb = const.tile([D, T], F32)
            nc.gpsimd.dma_start(out=ctabK_sb, in_=ctabK[:])
            stabK_sb = const.tile([D, T], F32)
            nc.gpsimd.dma_start(out=stabK_sb, in_=stabK[:])

            xtq = persist.tile([128, N_SLOTS, N_EC, SLOT_Q], F32R)
            for si in range(N_SLOTS):
                nc.gpsimd.dma_start(out=xtq[:, si], in_=xq_d[:, si])

            ctabQ_sb = const.tile([D, N_SLOTS * SLOT_Q], F32)
            nc.gpsimd.dma_start(out=ctabQ_sb, in_=ctabQ[:])
            stabQ_sb = const.tile([D, N_SLOTS * SLOT_Q], F32)
            nc.gpsimd.dma_start(out=stabQ_sb, in_=stabQ[:])
            mask_sb = const.tile([128, 4, SLOT_Q], F32)
            nc.gpsimd.dma_start(out=mask_sb, in_=masks[:])
            ones = const.tile([128, 128], F32R)
            nc.scalar.dma_start(out=ones, in_=ones_d[:])
            ident = const.tile([128, 128], F32)
            make_identity(nc, ident)

            kT_sb = persist.tile([D, T], F32R)
            qT_sb = persist.tile([D, N_SLOTS * SLOT_Q], F32R)
            v_nat = persist.tile([128, KV_CH, D], F32R)

            def rope(psum, width, ctab_ap, stab_ap, out_ap):
                raw = work.tile([128, width], F32, tag="raw")
                nc.scalar.copy(raw, psum)
                sw = work.tile([128, width], F32, tag="sw")
                s2 = raw.rearrange("(a b) f -> a b f", b=2)
                d2 = sw.rearrange("(a b) f -> a b f", b=2)
                nc.sync.dma_start(out=d2[:, 0, :], in_=s2[:, 1, :])
                nc.sync.dma_start(out=d2[:, 1, :], in_=s2[:, 0, :])
                t1 = work.tile([128, width], F32, tag="ropeA")
                nc.vector.tensor_mul(t1, psum, ctab_ap)
                t2 = work.tile([128, width], F32, tag="ropeB")
                nc.vector.tensor_mul(t2, sw, stab_ap)
                nc.vector.tensor_add(out_ap, t1, t2)

            # ---- K / V projections ----
            for tci in range(N_TC):
                cs = slice(tci * 512, (tci + 1) * 512)
                psk = pp.tile([128, 512], F32, tag="psk")
                psv = pp.tile([128, 512], F32, tag="psv")
                for ec in range(N_EC):
                    st, sp = ec == 0, ec == N_EC - 1
                    nc.tensor.matmul(psk, w_sb["k"][:, ec, :], xt[:, tci, ec, :], start=st, stop=sp)
                    nc.tensor.matmul(psv, w_sb["v"][:, ec, :], xt[:, tci, ec, :], start=st, stop=sp)
                rope(psk, 512, ctabK_sb[:, cs], stabK_sb[:, cs], kT_sb[:, cs])
                vt = work.tile([128, 512], F32, tag="vt")
                nc.vector.tensor_copy(vt, psv)
                for j in range(4):
                    pt = ps.tile([128, 128], F32, tag="s")
                    nc.tensor.transpose(pt, vt[:, j * 128:(j + 1) * 128], ident)
                    nc.scalar.copy(v_nat[:, tci * 4 + j, :], pt)

            # ---- Q projection ----
            for si in range(N_SLOTS):
                qs = slice(si * SLOT_Q, (si + 1) * SLOT_Q)
                psq = pp.tile([128, SLOT_Q], F32, tag="psk")
                for ec in range(N_EC):
                    nc.tensor.matmul(psq, w_sb["q"][:, ec, :], xtq[:, si, ec, :],
                                     start=ec == 0, stop=ec == N_EC - 1)
                rope(psq, SLOT_Q, ctabQ_sb[:, qs], stabQ_sb[:, qs], qT_sb[:, qs])

            # ---- Attention ----
            for si in range(N_SLOTS):
                qs = slice(si * SLOT_Q, (si + 1) * SLOT_Q)
                n_ch = 4 * (si + 1)
                pacc_av = pa.tile([128, SLOT_Q], F32, tag="pacc_av")
                pacc_d = pa.tile([128, SLOT_Q], F32, tag="pacc_d")
                for c in range(n_ch):
                    pss = ps.tile([128, SLOT_Q], F32, tag="s")
                    nc.tensor.matmul(pss, kT_sb[:, c * 128:(c + 1) * 128], qT_sb[:, qs],
                                     start=True, stop=True)
                    pT = work.tile([128, SLOT_Q], F32R, tag="pT", bufs=4)
                    nc.scalar.activation(out=pT, in_=pss,
                                         func=mybir.ActivationFunctionType.Exp, scale=float(SCALE))
                    j = c - (n_ch - 4)
                    if j >= 0:
                        nc.vector.tensor_mul(pT, pT, mask_sb[:, j, :])
                    st, sp = c == 0, c == n_ch - 1
                    nc.tensor.matmul(pacc_d, ones, pT, start=st, stop=sp)
                    nc.tensor.matmul(pacc_av, v_nat[:, c, :], pT, start=st, stop=sp)
                recip = work.tile([128, SLOT_Q], F32, tag="recip")
                nc.vector.reciprocal(recip, pacc_d)
                oT = work.tile([128, SLOT_Q], F32, tag="oT")
                nc.vector.tensor_mul(oT, pacc_av, recip)
                for j in range(2):
                    pt = pp.tile([128, 128], F32, tag="psv")
                    nc.tensor.transpose(pt, oT[:, j * 128:(j + 1) * 128], ident)
                    on = work.tile([128, 128], F32, tag="onat")
                    nc.scalar.copy(on, pt)
                    nc.sync.dma_start(out=out_d[si * SLOT_Q + j * 128: si * SLOT_Q + (j + 1) * 128, :],
                                      in_=on)
    nc.compile()
    return nc


_NC = None


def _get_nc():
    global _NC
    if _NC is None:
        _NC = _build_nc()
    return _NC


def _host_prep(embedding_word, w_Q, w_K, w_V):
    x = np.asarray(embedding_word, dtype=np.float32)
    w_Q = np.asarray(w_Q, dtype=np.float32)
    w_K = np.asarray(w_K, dtype=np.float32)
    w_V = np.asarray(w_V, dtype=np.float32)

    # packed weights: [p, ec, d] = W.T[ec*128+p, d]
    def pack_w(w):
        return np.ascontiguousarray(w.T.reshape(N_EC, 128, D).transpose(1, 0, 2))

    wq_p, wk_p, wv_p = pack_w(w_Q), pack_w(w_K), pack_w(w_V)

    # RoPE tables in [d, t] layout
    j = np.arange(D // 2, dtype=np.float64)
    freqs = 1.0 / THETA ** (2.0 * j / D)
    t = np.arange(T, dtype=np.float64)
    ang = np.outer(freqs, t)
    cos = np.cos(ang)
    sin = np.sin(ang)
    ctab = np.repeat(cos, 2, axis=0).astype(np.float32)
    stab = np.empty((D, T), dtype=np.float32)
    stab[0::2] = -sin
    stab[1::2] = sin

    qcols = {}
    for h in (0, 1):
        qcols[h] = np.concatenate([np.arange(512 * i + 256 * h, 512 * i + 256 * h + SLOT_Q)
                                   for i in range(N_SLOTS)])

    masks_h = {}
    for h in (0, 1):
        m = np.empty((4, 128, SLOT_Q), dtype=np.float32)
        for jj in range(4):
            xg, yg = np.meshgrid(np.arange(128), np.arange(SLOT_Q), indexing="ij")
            m[jj] = ((yg - xg) >= (128 * jj - 256 * h)).astype(np.float32)
        # pack to [p, j, y]
        masks_h[h] = np.ascontiguousarray(m.transpose(1, 0, 2))

    ones = np.ones((128, 128), dtype=np.float32)

    in_maps = []
    for c in range(N_CORES):
        b, h = c // 2, c % 2
        xT = x[b].T  # [E, T]
        # xt packed [p, tc, ec, t] = xT[ec*128+p, tc*512+t]
        xt_p = np.ascontiguousarray(
            xT.reshape(N_EC, 128, N_TC, 512).transpose(1, 2, 0, 3))
        xq = xT[:, qcols[h]]  # [E, 1024]
        xq_p = np.ascontiguousarray(
            xq.reshape(N_EC, 128, N_SLOTS, SLOT_Q).transpose(1, 2, 0, 3))
        in_maps.append({
            "xt": xt_p, "xq": xq_p,
            "wq": wq_p, "wk": wk_p, "wv": wv_p,
            "ctabK": ctab, "stabK": stab,
            "ctabQ": np.ascontiguousarray(ctab[:, qcols[h]]),
            "stabQ": np.ascontiguousarray(stab[:, qcols[h]]),
            "masks": masks_h[h],
            "ones": ones,
        })
    return in_maps


def _assemble(results):
    out = np.empty((B, T, D), dtype=np.float32)
    for c in range(N_CORES):
        b, h = c // 2, c % 2
        o = results[c]["out"]
        for i in range(N_SLOTS):
            out[b, 512 * i + 256 * h: 512 * i + 256 * h + SLOT_Q, :] = \
                o[i * SLOT_Q:(i + 1) * SLOT_Q, :]
    return out


def run(inputs, trace=False, tmpdir=None):
    nc = _get_nc()
    in_maps = _host_prep(**inputs)
    res = run_bass_kernel_spmd(nc, in_maps, list(range(N_CORES)), trace=trace, tmpdir=tmpdir)
    return _assemble(res.results), res


def kernel(embedding_word, w_Q, w_K, w_V):
    out, _ = run(dict(embedding_word=embedding_word, w_Q=w_Q, w_K=w_K, w_V=w_V))
    return out


# revision 7
# speedup vs baseline: 1.1191x; 1.0405x over previous
"""Causal single-head attention (B=4, T=2048, E=1024, D=128) on 8 TRN2 cores.

Sharding: core c = (b, h) with b = c // 2, h = c % 2. Each core handles batch b
and 4 query "slots" i=0..3: queries [512*i + 256*h, +256), keys [0, 512*(i+1))
(rectangularized causal; exact causality via data-driven multiplicative masks).
All cores run ONE identical bass program; per-core differences are expressed
purely via host-prepared DRAM input data.

Per core (all matmuls float32r):
  1. K/V projections over all 2048 tokens from host-pre-transposed, pre-packed
     xT tiles (contraction dim e on partitions, fully contiguous DMA).
  2. RoPE: raw k evicted to SBUF, partition-pair-swapped via 2 stride-2
     SBUF->SBUF DMAs, combined on DVE: k' = k*cosT + kswap*sinT.
  3. V^T -> V natural via PE transposes.
  4. Per slot: S^T chunk = k'^T_chunk.T @ q'^T -> exp on ACT -> mask mul ->
     ones-matmul denominator + AV matmul (separate PSUM banks) ->
     reciprocal+normalize on DVE -> PE transpose -> out.
"""

import sys

for _p in ("/opt/trn_rl_repo",):
    if _p not in sys.path:
        sys.path.insert(0, _p)

import numpy as np

import concourse.bacc as bacc
import concourse.mybir as mybir
import concourse.tile as tile
from concourse.bass_utils import run_bass_kernel_spmd
from concourse.masks import make_identity

F32 = mybir.dt.float32
F32R = mybir.dt.float32r

B, T, E, D = 4, 2048, 1024, 128
THETA = 10000.0
SCALE = 1.0 / np.sqrt(np.float32(D))
N_CORES = 8
N_SLOTS = 4
SLOT_Q = 256
KV_CH = T // 128
N_TC = T // 512
N_EC = E // 128


def _build_nc():
    nc = bacc.Bacc(None, target_bir_lowering=False, debug=False)

    # pre-packed inputs: [partition, ...] layouts, fully contiguous per row
    wk = nc.dram_tensor("wk", [128, N_EC, D], F32R, kind="ExternalInput")
    wv = nc.dram_tensor("wv", [128, N_EC, D], F32R, kind="ExternalInput")
    wq = nc.dram_tensor("wq", [128, N_EC, D], F32R, kind="ExternalInput")
    xt_d = nc.dram_tensor("xt", [128, N_TC, N_EC, 512], F32R, kind="ExternalInput")
    xq_d = nc.dram_tensor("xq", [128, N_SLOTS, N_EC, SLOT_Q], F32R, kind="ExternalInput")
    ctabK = nc.dram_tensor("ctabK", [D, T], F32, kind="ExternalInput")
    stabK = nc.dram_tensor("stabK", [D, T], F32, kind="ExternalInput")
    ctabQ = nc.dram_tensor("ctabQ", [D, N_SLOTS * SLOT_Q], F32, kind="ExternalInput")
    stabQ = nc.dram_tensor("stabQ", [D, N_SLOTS * SLOT_Q], F32, kind="ExternalInput")
    masks = nc.dram_tensor("masks", [128, 4, SLOT_Q], F32, kind="ExternalInput")
    ones_d = nc.dram_tensor("ones", [128, 128], F32R, kind="ExternalInput")
    out_d = nc.dram_tensor("out", [N_SLOTS * SLOT_Q, D], F32, kind="ExternalOutput")

    with tile.TileContext(nc) as tc:
        with (
            tc.tile_pool(name="const", bufs=1) as const,
            tc.tile_pool(name="persist", bufs=1) as persist,
            tc.tile_pool(name="work", bufs=2) as work,
            tc.tile_pool(name="pp", bufs=1, space="PSUM") as pp,
            tc.tile_pool(name="ps", bufs=4, space="PSUM") as ps,
            tc.tile_pool(name="pa", bufs=1, space="PSUM") as pa,
        ):
            # sync queue: wk, wv, xt0, k-tables, xt1..3 (kv-proj critical path)
            # scalar queue: wq, ones, xq, q-tables, masks (q/attention path)
            w_sb = {}
            for name, dram, eng in (("k", wk, nc.sync), ("v", wv, nc.sync), ("q", wq, nc.scalar)):
                t = const.tile([128, N_EC, D], F32R, tag=f"w_{name}")
                eng.dma_start(out=t, in_=dram[:])
                w_sb[name] = t

            ones = const.tile([128, 128], F32R)
            nc.scalar.dma_start(out=ones, in_=ones_d[:])

            xt = persist.tile([128, N_TC, N_EC, 512], F32R)
            nc.sync.dma_start(out=xt[:, 0], in_=xt_d[:, 0])
            ctabK_sb = const.tile([D, T], F32)
            nc.sync.dma_start(out=ctabK_sb, in_=ctabK[:])
            stabK_sb = const.tile([D, T], F32)
            nc.sync.dma_start(out=stabK_sb, in_=stabK[:])
            for tci in range(1, N_TC):
                nc.sync.dma_start(out=xt[:, tci], in_=xt_d[:, tci])

            xtq = persist.tile([128, N_SLOTS, N_EC, SLOT_Q], F32R)
            for si in range(N_SLOTS):
                nc.scalar.dma_start(out=xtq[:, si], in_=xq_d[:, si])

            ctabQ_sb = const.tile([D, N_SLOTS * SLOT_Q], F32)
            nc.scalar.dma_start(out=ctabQ_sb, in_=ctabQ[:])
            stabQ_sb = const.tile([D, N_SLOTS * SLOT_Q], F32)
            nc.scalar.dma_start(out=stabQ_sb, in_=stabQ[:])
            mask_sb = const.tile([128, 4, SLOT_Q], F32)
            nc.scalar.dma_start(out=mask_sb, in_=masks[:])
            ident = const.tile([128, 128], F32)
            make_identity(nc, ident)

            kT_sb = persist.tile([D, T], F32R)
            qT_sb = persist.tile([D, N_SLOTS * SLOT_Q], F32R)
            v_nat = persist.tile([128, KV_CH, D], F32R)

            def rope(psum, width, ctab_ap, stab_ap, out_ap):
                raw = work.tile([128, width], F32, tag="raw")
                nc.vector.tensor_copy(raw, psum)
                sw = work.tile([128, width], F32, tag="sw")
                s2 = raw.rearrange("(a b) f -> a b f", b=2)
                d2 = sw.rearrange("(a b) f -> a b f", b=2)
                nc.sync.dma_start(out=d2[:, 0, :], in_=s2[:, 1, :])
                nc.sync.dma_start(out=d2[:, 1, :], in_=s2[:, 0, :])
                t1 = work.tile([128, width], F32, tag="ropeA")
                nc.vector.tensor_mul(t1, psum, ctab_ap)
                t2 = work.tile([128, width], F32, tag="ropeB")
                nc.vector.tensor_mul(t2, sw, stab_ap)
                nc.vector.tensor_add(out_ap, t1, t2)

            # ---- K / V projections ----
            for tci in range(N_TC):
                cs = slice(tci * 512, (tci + 1) * 512)
                psk = pp.tile([128, 512], F32, tag="psk")
                psv = pp.tile([128, 512], F32, tag="psv")
                for ec in range(N_EC):
                    st, sp = ec == 0, ec == N_EC - 1
                    nc.tensor.matmul(psk, w_sb["k"][:, ec, :], xt[:, tci, ec, :], start=st, stop=sp)
                    nc.tensor.matmul(psv, w_sb["v"][:, ec, :], xt[:, tci, ec, :], start=st, stop=sp)
                rope(psk, 512, ctabK_sb[:, cs], stabK_sb[:, cs], kT_sb[:, cs])
                vt = work.tile([128, 512], F32, tag="vt")
                nc.vector.tensor_copy(vt, psv)
                for j in range(4):
                    pt = ps.tile([128, 128], F32, tag="s")
                    nc.tensor.transpose(pt, vt[:, j * 128:(j + 1) * 128], ident)
                    nc.scalar.copy(v_nat[:, tci * 4 + j, :], pt)

            # ---- Q projection ----
            for si in range(N_SLOTS):
                qs = slice(si * SLOT_Q, (si + 1) * SLOT_Q)
                psq = pp.tile([128, SLOT_Q], F32, tag="psk")
                for ec in range(N_EC):
                    nc.tensor.matmul(psq, w_sb["q"][:, ec, :], xtq[:, si, ec, :],
                                     start=ec == 0, stop=ec == N_EC - 1)
                rope(psq, SLOT_Q, ctabQ_sb[:, qs], stabQ_sb[:, qs], qT_sb[:, qs])

            # ---- Attention ----
            for si in range(N_SLOTS):
                qs = slice(si * SLOT_Q, (si + 1) * SLOT_Q)
                n_ch = 4 * (si + 1)
                pacc_av = pa.tile([128, SLOT_Q], F32, tag="pacc_av")
                pacc_d = pa.tile([128, SLOT_Q], F32, tag="pacc_d")
                for c in range(n_ch):
                    pss = ps.tile([128, SLOT_Q], F32, tag="s")
                    nc.tensor.matmul(pss, kT_sb[:, c * 128:(c + 1) * 128], qT_sb[:, qs],
                                     start=True, stop=True)
                    pT = work.tile([128, SLOT_Q], F32R, tag="pT", bufs=4)
                    nc.scalar.activation(out=pT, in_=pss,
                                         func=mybir.ActivationFunctionType.Exp, scale=float(SCALE))
                    j = c - (n_ch - 4)
                    if j >= 0:
                        nc.vector.tensor_mul(pT, pT, mask_sb[:, j, :])
                    st, sp = c == 0, c == n_ch - 1
                    nc.tensor.matmul(pacc_d, ones, pT, start=st, stop=sp)
                    nc.tensor.matmul(pacc_av, v_nat[:, c, :], pT, start=st, stop=sp)
                recip = work.tile([128, SLOT_Q], F32, tag="recip")
                nc.vector.reciprocal(recip, pacc_d)
                oT = work.tile([128, SLOT_Q], F32, tag="oT")
                nc.vector.tensor_mul(oT, pacc_av, recip)
                for j in range(2):
                    pt = pp.tile([128, 128], F32, tag="psv")
                    nc.tensor.transpose(pt, oT[:, j * 128:(j + 1) * 128], ident)
                    on = work.tile([128, 128], F32, tag="onat")
                    nc.scalar.copy(on, pt)
                    nc.sync.dma_start(out=out_d[si * SLOT_Q + j * 128: si * SLOT_Q + (j + 1) * 128, :],
                                      in_=on)
    nc.compile()
    return nc


_NC = None


def _get_nc():
    global _NC
    if _NC is None:
        _NC = _build_nc()
    return _NC


def _host_prep(embedding_word, w_Q, w_K, w_V):
    x = np.asarray(embedding_word, dtype=np.float32)
    w_Q = np.asarray(w_Q, dtype=np.float32)
    w_K = np.asarray(w_K, dtype=np.float32)
    w_V = np.asarray(w_V, dtype=np.float32)

    # packed weights: [p, ec, d] = W.T[ec*128+p, d]
    def pack_w(w):
        return np.ascontiguousarray(w.T.reshape(N_EC, 128, D).transpose(1, 0, 2))

    wq_p, wk_p, wv_p = pack_w(w_Q), pack_w(w_K), pack_w(w_V)

    # RoPE tables in [d, t] layout
    j = np.arange(D // 2, dtype=np.float64)
    freqs = 1.0 / THETA ** (2.0 * j / D)
    t = np.arange(T, dtype=np.float64)
    ang = np.outer(freqs, t)
    cos = np.cos(ang)
    sin = np.sin(ang)
    ctab = np.repeat(cos, 2, axis=0).astype(np.float32)
    stab = np.empty((D, T), dtype=np.float32)
    stab[0::2] = -sin
    stab[1::2] = sin

    qcols = {}
    for h in (0, 1):
        qcols[h] = np.concatenate([np.arange(512 * i + 256 * h, 512 * i + 256 * h + SLOT_Q)
                                   for i in range(N_SLOTS)])

    masks_h = {}
    for h in (0, 1):
        m = np.empty((4, 128, SLOT_Q), dtype=np.float32)
        for jj in range(4):
            xg, yg = np.meshgrid(np.arange(128), np.arange(SLOT_Q), indexing="ij")
            m[jj] = ((yg - xg) >= (128 * jj - 256 * h)).astype(np.float32)
        # pack to [p, j, y]
        masks_h[h] = np.ascontiguousarray(m.transpose(1, 0, 2))

    ones = np.ones((128, 128), dtype=np.float32)

    in_maps = []
    for c in range(N_CORES):
        b, h = c // 2, c % 2
        xT = x[b].T  # [E, T]
        # xt packed [p, tc, ec, t] = xT[ec*128+p, tc*512+t]
        xt_p = np.ascontiguousarray(
            xT.reshape(N_EC, 128, N_TC, 512).transpose(1, 2, 0, 3))
        xq = xT[:, qcols[h]]  # [E, 1024]
        xq_p = np.ascontiguousarray(
            xq.reshape(N_EC, 128, N_SLOTS, SLOT_Q).transpose(1, 2, 0, 3))
        in_maps.append({
            "xt": xt_p, "xq": xq_p,
            "wq": wq_p, "wk": wk_p, "wv": wv_p,
            "ctabK": ctab, "stabK": stab,
            "ctabQ": np.ascontiguousarray(ctab[:, qcols[h]]),
            "stabQ": np.ascontiguousarray(stab[:, qcols[h]]),
            "masks": masks_h[h],
            "ones": ones,
        })
    return in_maps


def _assemble(results):
    out = np.empty((B, T, D), dtype=np.float32)
    for c in range(N_CORES):
        b, h = c // 2, c % 2
        o = results[c]["out"]
        for i in range(N_SLOTS):
            out[b, 512 * i + 256 * h: 512 * i + 256 * h + SLOT_Q, :] = \
                o[i * SLOT_Q:(i + 1) * SLOT_Q, :]
    return out


def run(inputs, trace=False, tmpdir=None):
    nc = _get_nc()
    in_maps = _host_prep(**inputs)
    res = run_bass_kernel_spmd(nc, in_maps, list(range(N_CORES)), trace=trace, tmpdir=tmpdir)
    return _assemble(res.results), res


def kernel(embedding_word, w_Q, w_K, w_V):
    out, _ = run(dict(embedding_word=embedding_word, w_Q=w_Q, w_K=w_K, w_V=w_V))
    return out
